# revision 1
# baseline (speedup 1.0000x reference)
"""NoisyTopKRouter Trainium2 kernel.

Full inputs in, full outputs out; shards tokens across 8 NeuronCores.

Per-core dataflow (N_SH=2048 tokens, D=1024, E=64):
  host: xT = x_shard.T (contiguous), Wcat = [route_w; noise_w].T [D, 2E]
  device, per 512-token group:
    psum[2E, 512]  = sum_c WcatT_c.T @ xT_c          (8 fp32 matmuls, K=128)
    lt             = psum + bias_cat                  (DVE, psum->sbuf)
    psumT[512, 2E] = PE transpose of lt               (4x 128x128)
    noise_scale    = ln(1+exp(noise half))            (ACT, exp/ln one table set)
    noisy          = route half + eps * noise_scale   (DVE)
    top2 via DVE max/max_index; probs = exp(noisy) * (noisy>=s2) / (e^s1+e^s2)
"""
import numpy as np

N, D, E = 16384, 1024, 64
NCORES = 8
N_SH = N // NCORES        # 2048 tokens per core
GSZ = 512                 # tokens per group
NG = N_SH // GSZ          # 4 groups
NSUB = GSZ // 128         # 4 subtiles per group
NCH = D // 128            # 8 contraction chunks
EC = 2 * E                # 128 = route|noise concatenated

_compiled = None


def _build():
    import concourse.bacc as bacc
    import concourse.mybir as mybir
    from concourse.tile import TileContext
    from concourse.masks import make_identity

    F32 = mybir.dt.float32
    U32 = mybir.dt.uint32
    AF = mybir.ActivationFunctionType
    ALU = mybir.AluOpType

    nc = bacc.Bacc(None, target_bir_lowering=False, debug=False,
                   num_devices=NCORES)
    xt_in = nc.dram_tensor("xt", [D, N_SH], F32, kind="ExternalInput").ap()
    wc_in = nc.dram_tensor("wc", [D, EC], F32, kind="ExternalInput").ap()
    bc_in = nc.dram_tensor("bc", [EC, 1], F32, kind="ExternalInput").ap()
    eps_in = nc.dram_tensor("eps", [N_SH, E], F32, kind="ExternalInput").ap()
    probs_out = nc.dram_tensor("probs", [N_SH, E], F32,
                               kind="ExternalOutput").ap()
    idx_out = nc.dram_tensor("idx", [N_SH, 2], U32, kind="ExternalOutput").ap()

    with TileContext(nc) as tc:
        with (
            tc.tile_pool(name="const", bufs=1) as cpool,
            tc.tile_pool(name="work", bufs=2) as pool,
            tc.tile_pool(name="xgp", bufs=3) as xpool,
            tc.tile_pool(name="psmm", bufs=2, space="PSUM") as psmm,
            tc.tile_pool(name="pstr", bufs=2, space="PSUM") as pstr,
        ):
            wc = cpool.tile([128, NCH, EC], F32)
            nc.sync.dma_start(out=wc[:], in_=wc_in.rearrange(
                "(c p) m -> p c m", p=128))
            bc = cpool.tile([128, 1], F32)
            nc.sync.dma_start(out=bc[:], in_=bc_in)
            epsb = cpool.tile([128, N_SH // 128, E], F32)
            nc.sync.dma_start(out=epsb[:], in_=eps_in.rearrange(
                "(t p) e -> p t e", p=128))
            ident = cpool.tile([128, 128], F32)
            make_identity(nc, ident[:])

            for g in range(NG):
                xg = xpool.tile([128, NCH, GSZ], F32, tag="xg")
                nc.sync.dma_start(
                    out=xg[:],
                    in_=xt_in[:, g * GSZ:(g + 1) * GSZ].rearrange(
                        "(c p) n -> p c n", p=128))

                mm = psmm.tile([EC, GSZ], F32, tag="mm")
                for c in range(NCH):
                    nc.tensor.matmul(mm[:], wc[:, c, :], xg[:, c, :],
                                     start=(c == 0), stop=(c == NCH - 1))

                lt = pool.tile([EC, GSZ], F32, tag="lt")
                nc.vector.tensor_scalar(lt[:], mm[:], bc[:, 0:1], None,
                                        op0=ALU.add)

                tr = pstr.tile([128, NSUB, EC], F32, tag="tr")
                for t in range(NSUB):
                    nc.tensor.transpose(tr[:, t], lt[:, t * 128:(t + 1) * 128],
                                        ident[:])
                rtv = tr[:, :, 0:E]      # [128, NSUB, E] route logits
                nsv = tr[:, :, E:EC]     # [128, NSUB, E] noise logits

                # noise_scale = ln(1 + exp(nsv))
                ex1 = pool.tile([128, NSUB, E], F32, tag="ex1")
                nc.scalar.activation(ex1[:], nsv, AF.Exp)
                nc.scalar.add(ex1[:], ex1[:], 1.0)
                ns = pool.tile([128, NSUB, E], F32, tag="ns")
                nc.scalar.activation(ns[:], ex1[:], AF.Ln)

                nm = pool.tile([128, NSUB, E], F32, tag="nm")
                nc.vector.tensor_mul(nm[:], epsb[:, g * NSUB:(g + 1) * NSUB, :],
                                     ns[:])
                noisy = pool.tile([128, NSUB, E], F32, tag="noisy")
                nc.vector.tensor_add(noisy[:], rtv, nm[:])

                mx8 = pool.tile([128, NSUB, 8], F32, tag="mx8")
                ix8 = pool.tile([128, NSUB, 8], U32, tag="ix8")
                for t in range(NSUB):
                    nc.vector.max(out=mx8[:, t], in_=noisy[:, t])
                    nc.vector.max_index(ix8[:, t], mx8[:, t], noisy[:, t])

                e8 = pool.tile([128, NSUB, 8], F32, tag="e8")
                nc.scalar.activation(e8[:], mx8[:], AF.Exp)
                z4 = pool.tile([128, NSUB], F32, tag="z4")
                nc.vector.tensor_add(z4[:], e8[:, :, 0], e8[:, :, 1])
                rz4 = pool.tile([128, NSUB], F32, tag="rz4")
                nc.vector.reciprocal(rz4[:], z4[:])

                exv = pool.tile([128, NSUB, E], F32, tag="exv")
                nc.scalar.activation(exv[:], noisy[:], AF.Exp)
                mrz = pool.tile([128, NSUB, E], F32, tag="mrz")
                for t in range(NSUB):
                    nc.vector.tensor_scalar(mrz[:, t], noisy[:, t],
                                            mx8[:, t, 1:2], rz4[:, t:t + 1],
                                            op0=ALU.is_ge, op1=ALU.mult)
                prb = pool.tile([128, NSUB, E], F32, tag="prb")
                nc.vector.tensor_mul(prb[:], exv[:], mrz[:])

                nc.scalar.dma_start(
                    out=probs_out[g * GSZ:(g + 1) * GSZ, :].rearrange(
                        "(t p) e -> p t e", p=128),
                    in_=prb[:])
                nc.scalar.dma_start(
                    out=idx_out[g * GSZ:(g + 1) * GSZ, :].rearrange(
                        "(t p) k -> p t k", p=128),
                    in_=ix8[:, :, 0:2])

    nc.compile()
    return nc


def _get_compiled():
    global _compiled
    if _compiled is None:
        _compiled = _build()
    return _compiled


def kernel(x, route_w, route_b, noise_w, noise_b, eps):
    from concourse.bass_utils import run_bass_kernel_spmd

    x = np.ascontiguousarray(np.asarray(x, dtype=np.float32))
    route_w = np.asarray(route_w, dtype=np.float32)
    route_b = np.asarray(route_b, dtype=np.float32)
    noise_w = np.asarray(noise_w, dtype=np.float32)
    noise_b = np.asarray(noise_b, dtype=np.float32)
    eps = np.ascontiguousarray(np.asarray(eps, dtype=np.float32))

    wc = np.ascontiguousarray(
        np.concatenate([route_w, noise_w], axis=0).T)          # [D, 2E]
    bc = np.concatenate([route_b, noise_b]).reshape(EC, 1)
    bc = np.ascontiguousarray(bc)

    in_maps = []
    for c in range(NCORES):
        sl = slice(c * N_SH, (c + 1) * N_SH)
        in_maps.append({
            "xt": np.ascontiguousarray(x[sl].T),
            "wc": wc,
            "bc": bc,
            "eps": np.ascontiguousarray(eps[sl]),
        })

    nc = _get_compiled()
    res = run_bass_kernel_spmd(nc, in_maps, list(range(NCORES)))

    probs = np.concatenate([res.results[c]["probs"] for c in range(NCORES)], 0)
    idx = np.concatenate([res.results[c]["idx"] for c in range(NCORES)], 0)
    return probs, idx.view(np.int32)


# revision 3
# speedup vs baseline: 1.1234x; 1.1234x over previous
"""NoisyTopKRouter Trainium2 kernel.

Full inputs in, full outputs out; shards tokens across 8 NeuronCores.

Per-core dataflow (N_SH=2048 tokens, D=1024, E=64):
  host: xT = x_shard.T, epsT = eps_shard.T, Wcat = [route_w; noise_w].T
  device, per 512-token group g:
    psum[2E, 512] = sum_c WcatT_c.T @ xT_c        (8 fp32 matmuls, K=128)
    ns    = ln(1 + exp(psum[E:2E] + noise_b))      (ACT exp/ln, bias-folded)
    noisyT= (psum[0:E] + route_b) + epsT * ns      (DVE, [E, 512])
    psumT[512 tok, E] = PE transpose of noisyT     (4x [64,128] transposes)
    top2 via DVE max/max_index on psumT; probs = exp(noisy)*(noisy>=s2)/(e^s1+e^s2)
"""
import numpy as np

N, D, E = 16384, 1024, 64
NCORES = 8
N_SH = N // NCORES        # 2048 tokens per core
GSZ = 512                 # tokens per group
NG = N_SH // GSZ          # 4 groups
NSUB = GSZ // 128         # 4 subtiles per group
NCH = D // 128            # 8 contraction chunks
EC = 2 * E                # 128 = route|noise concatenated

_compiled = None


def _build():
    import concourse.bacc as bacc
    import concourse.mybir as mybir
    from concourse.tile import TileContext
    from concourse.masks import make_identity

    F32 = mybir.dt.float32
    U32 = mybir.dt.uint32
    AF = mybir.ActivationFunctionType
    ALU = mybir.AluOpType

    nc = bacc.Bacc(None, target_bir_lowering=False, debug=False,
                   num_devices=NCORES)
    xt_in = nc.dram_tensor("xt", [D, N_SH], F32, kind="ExternalInput").ap()
    wc_in = nc.dram_tensor("wc", [D, EC], F32, kind="ExternalInput").ap()
    bc_in = nc.dram_tensor("bc", [EC, 1], F32, kind="ExternalInput").ap()
    epst_in = nc.dram_tensor("epst", [E, N_SH], F32, kind="ExternalInput").ap()
    probs_out = nc.dram_tensor("probs", [N_SH, E], F32,
                               kind="ExternalOutput").ap()
    idx_out = nc.dram_tensor("idx", [N_SH, 2], U32, kind="ExternalOutput").ap()

    with TileContext(nc) as tc:
        with (
            tc.tile_pool(name="const", bufs=1) as cpool,
            tc.tile_pool(name="work", bufs=2) as pool,
            tc.tile_pool(name="xgp", bufs=3) as xpool,
            tc.tile_pool(name="psmm", bufs=3, space="PSUM") as psmm,
            tc.tile_pool(name="pstr", bufs=3, space="PSUM") as pstr,
        ):
            # weights first on the sync (HWDGE/SP) ring so matmuls start early
            wc = cpool.tile([128, NCH, EC], F32)
            nc.sync.dma_start(out=wc[:], in_=wc_in.rearrange(
                "(c p) m -> p c m", p=128))
            # small constants + eps on the gpsimd (SWDGE) ring
            bc = cpool.tile([EC, 1], F32)
            nc.gpsimd.dma_start(out=bc[:], in_=bc_in)
            epst = cpool.tile([E, NG, GSZ], F32)
            nc.gpsimd.dma_start(out=epst[:], in_=epst_in.rearrange(
                "e (g n) -> e g n", g=NG))
            ident = cpool.tile([128, 128], F32)
            make_identity(nc, ident[:])

            for g in range(NG):
                xg = xpool.tile([128, NCH, GSZ], F32, tag="xg")
                # per-chunk DMAs so chunk-0 matmul starts after ~256KB
                for c in range(NCH):
                    nc.sync.dma_start(
                        out=xg[:, c, :],
                        in_=xt_in[c * 128:(c + 1) * 128,
                                  g * GSZ:(g + 1) * GSZ])

                mm = psmm.tile([EC, GSZ], F32, tag="mm")
                for c in range(NCH):
                    nc.tensor.matmul(mm[:], wc[:, c, :], xg[:, c, :],
                                     start=(c == 0), stop=(c == NCH - 1))

                # noise_scale = ln(1 + exp(noise_logits + noise_b)), [E, GSZ]
                ex1 = pool.tile([E, GSZ], F32, tag="ex1")
                nc.scalar.activation(ex1[:], mm[E:EC, :], AF.Exp,
                                     bias=bc[E:EC, 0:1])
                ns = pool.tile([E, GSZ], F32, tag="ns")
                nc.scalar.activation(ns[:], ex1[:], AF.Ln, bias=1.0)

                # noisyT = (route_logits + route_b) + epsT * ns, [E, GSZ]
                nm = pool.tile([E, GSZ], F32, tag="nm")
                nc.vector.tensor_mul(nm[:], epst[:, g, :], ns[:])
                rt = pool.tile([E, GSZ], F32, tag="rt")
                nc.vector.tensor_scalar(rt[:], mm[0:E, :], bc[0:E, 0:1], None,
                                        op0=ALU.add)
                noisyT = pool.tile([E, GSZ], F32, tag="noisyT")
                nc.vector.tensor_add(noisyT[:], rt[:], nm[:])

                # transpose noisyT -> [GSZ tok, E] in psum
                tr = pstr.tile([128, NSUB, E], F32, tag="tr")
                for t in range(NSUB):
                    nc.tensor.transpose(tr[:, t],
                                        noisyT[:, t * 128:(t + 1) * 128],
                                        ident[0:E, 0:E])

                mx8 = pool.tile([128, NSUB, 8], F32, tag="mx8")
                ix8 = pool.tile([128, NSUB, 8], U32, tag="ix8")
                for t in range(NSUB):
                    nc.vector.max(out=mx8[:, t], in_=tr[:, t])
                    nc.vector.max_index(ix8[:, t], mx8[:, t], tr[:, t])

                e8 = pool.tile([128, NSUB, 8], F32, tag="e8")
                nc.scalar.activation(e8[:], mx8[:], AF.Exp)
                z4 = pool.tile([128, NSUB], F32, tag="z4")
                nc.vector.tensor_add(z4[:], e8[:, :, 0], e8[:, :, 1])
                rz4 = pool.tile([128, NSUB], F32, tag="rz4")
                nc.vector.reciprocal(rz4[:], z4[:])

                exv = pool.tile([128, NSUB, E], F32, tag="exv")
                nc.scalar.activation(exv[:], tr[:], AF.Exp)
                mrz = pool.tile([128, NSUB, E], F32, tag="mrz")
                for t in range(NSUB):
                    nc.vector.tensor_scalar(mrz[:, t], tr[:, t],
                                            mx8[:, t, 1:2], rz4[:, t:t + 1],
                                            op0=ALU.is_ge, op1=ALU.mult)
                prb = pool.tile([128, NSUB, E], F32, tag="prb")
                nc.vector.tensor_mul(prb[:], exv[:], mrz[:])

                nc.scalar.dma_start(
                    out=probs_out[g * GSZ:(g + 1) * GSZ, :].rearrange(
                        "(t p) e -> p t e", p=128),
                    in_=prb[:])
                nc.scalar.dma_start(
                    out=idx_out[g * GSZ:(g + 1) * GSZ, :].rearrange(
                        "(t p) k -> p t k", p=128),
                    in_=ix8[:, :, 0:2])

    nc.compile()
    return nc


def _get_compiled():
    global _compiled
    if _compiled is None:
        _compiled = _build()
    return _compiled


def make_in_maps(x, route_w, route_b, noise_w, noise_b, eps):
    x = np.ascontiguousarray(np.asarray(x, dtype=np.float32))
    eps = np.ascontiguousarray(np.asarray(eps, dtype=np.float32))
    wc = np.ascontiguousarray(
        np.concatenate([np.asarray(route_w, dtype=np.float32),
                        np.asarray(noise_w, dtype=np.float32)], axis=0).T)
    bc = np.ascontiguousarray(
        np.concatenate([np.asarray(route_b, dtype=np.float32),
                        np.asarray(noise_b, dtype=np.float32)]).reshape(EC, 1))
    in_maps = []
    for c in range(NCORES):
        sl = slice(c * N_SH, (c + 1) * N_SH)
        in_maps.append({
            "xt": np.ascontiguousarray(x[sl].T),
            "wc": wc,
            "bc": bc,
            "epst": np.ascontiguousarray(eps[sl].T),
        })
    return in_maps


def kernel(x, route_w, route_b, noise_w, noise_b, eps):
    from concourse.bass_utils import run_bass_kernel_spmd

    in_maps = make_in_maps(x, route_w, route_b, noise_w, noise_b, eps)
    nc = _get_compiled()
    res = run_bass_kernel_spmd(nc, in_maps, list(range(NCORES)))

    probs = np.concatenate([res.results[c]["probs"] for c in range(NCORES)], 0)
    idx = np.concatenate([res.results[c]["idx"] for c in range(NCORES)], 0)
    return probs, idx.view(np.int32)
